# revision 8
# baseline (speedup 1.0000x reference)
"""Trainium2 Bass kernel for BasicAttentionModule (pooled attention + residual).

Computation (per sample): 8x8 avg-pool -> 1x1-conv q/k/v over 1024 tokens ->
softmax attention -> nearest 8x upsample -> residual add.

Sharding: 2 cores per sample (batch 4 x 8 cores); each core owns 128 of the
256 rows.  Per core:
  phase 1  pool own half: H-direction sum rides on SWDGE accumulate-DMAs
           (1 MiB each, 2048 elem/partition CCE cap), W-direction on a DVE
           pairwise tree.
  gather   pooled tokens exchanged with the pair core via AllGather.
  attn     fp32 on the PE, tokens-on-partitions layout (no transposes):
           E[m,n] = exp(energy^T), denominator via all-ones matmul,
           out_small = vT.T @ E normalized on evacuation.
  phase 3  features stream in again on HWDGE (prefetched during attn),
           DVE adds the upsampled attention via a stride-0 broadcast
           operand, HWDGE stores.
"""

import numpy as np

import concourse.bass as bass
import concourse.mybir as mybir
import concourse.tile as tile
from concourse.bass_utils import run_bass_kernel_spmd

F32 = mybir.dt.float32

B, C, H, W = 4, 256, 256, 256
S = 8                      # pool stride
KCH = 32                   # key channels
N_CORES = 8
HH = H // 2                # 128 rows per core
HP, WP = HH // S, W // S   # 16 x 32 pooled grid per core half
NT = HP * WP               # 512 tokens owned per core
NTOK = 2 * NT              # 1024 tokens per sample
CCH = C // 128             # 2 channel chunks
NJ = NTOK // 128           # 8 token chunks (m on partitions)

_CACHE: dict = {}


def _split_multi_waits(nc):
    """walrus in this container accepts at most ONE sync-wait per
    instruction; hoist extra waits onto inserted NoOps (same engine,
    right before the instruction -> identical semantics)."""
    import json

    d = json.loads(mybir.module_to_json_string(nc.m))
    n = 0
    for fn in d["functions"]:
        for bb in fn["blocks"]:
            out = []
            for inst in bb.get("instructions", []):
                si = inst.get("sync_info")
                waits = (si or {}).get("on_wait") or []
                if len(waits) > 1:
                    for w in waits[:-1]:
                        n += 1
                        out.append({
                            "debug": inst.get("debug", 0),
                            "engine": inst["engine"],
                            "ins": [], "outs": [],
                            "name": f"I-wsplit-{n}",
                            "opcode": "NoOp",
                            "sync_info": {"on_update": [], "on_wait": [w]},
                        })
                    si["on_wait"] = [waits[-1]]
                out.append(inst)
            bb["instructions"] = out
    nc.m = mybir.module_from_json_string(json.dumps(d))
    return n


def _build(split_waits=True):
    nc = bass.Bass(num_devices=N_CORES)

    xh = nc.declare_dram_parameter("xh", [C, HH, W], F32, isOutput=False)
    qw = nc.declare_dram_parameter("qw", [CCH, 128, KCH], F32, isOutput=False)
    kw = nc.declare_dram_parameter("kw", [CCH, 128, KCH], F32, isOutput=False)
    vw = nc.declare_dram_parameter("vw", [CCH, 128, C], F32, isOutput=False)
    qb = nc.declare_dram_parameter("qb", [KCH], F32, isOutput=False)
    kb = nc.declare_dram_parameter("kb", [KCH], F32, isOutput=False)
    vb = nc.declare_dram_parameter("vb", [C], F32, isOutput=False)
    out = nc.declare_dram_parameter("out", [C, HH, W], F32, isOutput=True)

    with tile.TileContext(nc) as tc:
        with (
            tc.tile_pool(name="const", bufs=1) as constp,
            tc.tile_pool(name="ph", bufs=3) as php,
            tc.tile_pool(name="wtree", bufs=2) as wtp,
            tc.tile_pool(name="attn", bufs=1) as attnp,
            tc.tile_pool(name="feat", bufs=6) as featp,
            tc.tile_pool(name="up", bufs=4) as upp,
            tc.tile_pool(name="pqk", bufs=1, space="PSUM") as pqk,
            tc.tile_pool(name="pe", bufs=1, space="PSUM") as pep,
            tc.tile_pool(name="pacc", bufs=1, space="PSUM") as pacc,
            tc.tile_pool(name="dram", bufs=1, space="DRAM") as dram,
        ):
            # ---- constants / weights ----
            qw_sb = [constp.tile([128, KCH], F32, name=f"qw{k}") for k in range(CCH)]
            kw_sb = [constp.tile([128, KCH], F32, name=f"kw{k}") for k in range(CCH)]
            vw_sb = [constp.tile([128, C], F32, name=f"vw{k}") for k in range(CCH)]
            qb_sb = constp.tile([KCH, 1], F32, name="qb")
            kb_sb = constp.tile([KCH, 1], F32, name="kb")
            vb_row = constp.tile([1, C], F32, name="vb")
            for k in range(CCH):
                nc.scalar.dma_start(out=qw_sb[k][:], in_=qw[k])
                nc.scalar.dma_start(out=kw_sb[k][:], in_=kw[k])
                nc.scalar.dma_start(out=vw_sb[k][:], in_=vw[k])
            nc.scalar.dma_start(out=qb_sb[:], in_=qb[:])
            nc.scalar.dma_start(out=kb_sb[:], in_=kb[:])
            nc.scalar.dma_start(out=vb_row[:], in_=vb[:])
            ones_col = constp.tile([1, 128], F32, name="ones_col")
            ones128 = constp.tile([128, 128], F32, name="ones128")
            nc.vector.memset(ones_col[:], 1.0)
            nc.vector.memset(ones128[:], 1.0)

            xf_own = [constp.tile([128, NT], F32, name=f"xfo{k}") for k in range(CCH)]
            xf_full = [constp.tile([128, NTOK], F32, name=f"xff{k}") for k in range(CCH)]

            # ---- phase 1: pooling ----
            # H-sum: 8 accumulate-DMAs onto a [128, 8, 256] group
            # (= 2048 elem/partition, the CCE accumulate cap).
            for k in range(CCH):
                cs = slice(k * 128, (k + 1) * 128)
                for g in range(2):
                    ph = php.tile([128, 8, W], F32, tag="ph", name="ph")
                    r0 = g * 64
                    for r in range(S):
                        nc.gpsimd.dma_start(
                            out=ph[:],
                            in_=xh[cs, r0 + r : r0 + 64 : S, :],
                            accum_op=(mybir.AluOpType.bypass if r == 0
                                      else mybir.AluOpType.add),
                        )
                    # W-sum: pairwise tree down to 32 pooled columns
                    a = wtp.tile([128, 8, W // 2], F32, tag="wta", name="wta")
                    nc.vector.tensor_add(a[:], ph[:, :, 0::2], ph[:, :, 1::2])
                    b = wtp.tile([128, 8, W // 4], F32, tag="wtb", name="wtb")
                    nc.vector.tensor_add(b[:], a[:, :, 0::2], a[:, :, 1::2])
                    dst = xf_own[k][:, g * 8 * WP : (g + 1) * 8 * WP]
                    dst = dst.rearrange("c (i wp) -> c i wp", i=8)
                    nc.vector.tensor_add(dst, b[:, :, 0::2], b[:, :, 1::2])

            # ---- pair AllGather of pooled tokens ----
            cc_in = dram.tile([C, NT], F32, name="cc_in")
            cc_out = dram.tile([2, C, NT], F32, name="cc_out")
            for k in range(CCH):
                nc.gpsimd.dma_start(out=cc_in[k * 128:(k + 1) * 128, :],
                                    in_=xf_own[k][:])
            nc.gpsimd.collective_compute(
                "AllGather", mybir.AluOpType.bypass,
                replica_groups=[[0, 1], [2, 3], [4, 5], [6, 7]],
                ins=[cc_in.opt()], outs=[cc_out.opt()],
            )
            for k in range(CCH):
                src = cc_out[:, k * 128:(k + 1) * 128, :].rearrange("g c t -> c g t")
                dstf = xf_full[k].rearrange("c (g t) -> c g t", g=2)
                nc.scalar.dma_start(out=dstf, in_=src)

            # ---- q / k projections (q only needs own tokens) ----
            q_sb = attnp.tile([KCH, NT], F32, name="q_sb")
            psum_q = pqk.tile([KCH, NT], F32, tag="qk", name="psum_q")
            for k in range(CCH):
                nc.tensor.matmul(psum_q[:], qw_sb[k][:], xf_own[k][:],
                                 start=(k == 0), stop=(k == CCH - 1))
            nc.vector.tensor_scalar_add(q_sb[:], psum_q[:], qb_sb[:])

            k_sb = attnp.tile([KCH, NTOK], F32, name="k_sb")
            psum_k = pqk.tile([KCH, NTOK], F32, tag="qk", name="psum_k")
            for mh in range(2):
                ms = slice(mh * NT, (mh + 1) * NT)
                for k in range(CCH):
                    nc.tensor.matmul(psum_k[:, ms], kw_sb[k][:],
                                     xf_full[k][:, ms],
                                     start=(k == 0), stop=(k == CCH - 1))
            nc.vector.tensor_scalar_add(k_sb[:], psum_k[:], kb_sb[:])

            # ---- v^T (tokens on partitions), bias via K=1 ones matmul ----
            vt_sb = [attnp.tile([128, C], F32, name=f"vt{j}") for j in range(NJ)]
            for j in range(NJ):
                js = slice(j * 128, (j + 1) * 128)
                psum_vt = pep.tile([128, C], F32, tag="pvt", name="psum_vt")
                for k in range(CCH):
                    nc.tensor.matmul(psum_vt[:], xf_full[k][:, js], vw_sb[k][:],
                                     start=(k == 0), stop=False)
                nc.tensor.matmul(psum_vt[:], ones_col[:, :], vb_row[:],
                                 start=False, stop=True)
                nc.vector.tensor_copy(vt_sb[j][:], psum_vt[:])

            # ---- energies (E = exp(energy^T)) ----
            e_sb = [attnp.tile([128, NT], F32, name=f"e{j}") for j in range(NJ)]
            for j in range(NJ):
                js = slice(j * 128, (j + 1) * 128)
                psum_e = pep.tile([128, NT], F32, tag="pe", bufs=2, name="psum_e")
                nc.tensor.matmul(psum_e[:], k_sb[:, js], q_sb[:],
                                 start=True, stop=True)
                nc.scalar.activation(e_sb[j][:], psum_e[:],
                                     mybir.ActivationFunctionType.Exp)

            # ---- softmax denominator (all-ones matmul broadcasts it) ----
            recip = attnp.tile([128, NT], F32, name="recip")
            psum_den = pacc.tile([128, NT], F32, tag="den", name="psum_den")
            for j in range(NJ):
                nc.tensor.matmul(psum_den[:], ones128[:], e_sb[j][:],
                                 start=(j == 0), stop=(j == NJ - 1))
            nc.vector.reciprocal(recip[:], psum_den[:])

            # ---- attention output on the pooled grid, normalized ----
            os_sb = [attnp.tile([128, NT], F32, name=f"os{k}") for k in range(CCH)]
            for k in range(CCH):
                psum_os = pacc.tile([128, NT], F32, tag="pos", bufs=2,
                                    name="psum_os")
                for j in range(NJ):
                    nc.tensor.matmul(psum_os[:], vt_sb[j][:, k * 128:(k + 1) * 128],
                                     e_sb[j][:], start=(j == 0), stop=(j == NJ - 1))
                nc.vector.tensor_mul(os_sb[k][:], psum_os[:], recip[:])

            # ---- phase 3: second feature read + broadcast-add + store ----
            # feat loads sit early in the sync queue -> they prefetch while
            # the gather/attention runs; the DVE add reads the upsampled
            # attention values through a stride-0 AP (no materialization).
            for k in range(CCH):
                cs = slice(k * 128, (k + 1) * 128)
                for t in range(HP):
                    rs = slice(t * S, (t + 1) * S)
                    feat = featp.tile([128, S, W], F32, tag="feat", name="feat")
                    nc.sync.dma_start(out=feat[:], in_=xh[cs, rs, :])
                    up = upp.tile([128, S, W], F32, tag="up", name="up")
                    src = bass.AP(os_sb[k].tensor, os_sb[k].offset + t * WP,
                                  [list(os_sb[k].ap[0]),
                                   [0, S], [1, WP], [0, S]])
                    f4 = feat.rearrange("c h (wp wr) -> c h wp wr", wr=S)
                    u4 = up.rearrange("c h (wp wr) -> c h wp wr", wr=S)
                    nc.vector.tensor_add(u4, f4, src)
                    nc.scalar.dma_start(out=out[cs, rs, :], in_=up[:])

    if split_waits:
        _split_multi_waits(nc)
    return nc


def _get_nc():
    if "nc" not in _CACHE:
        _CACHE["nc"] = _build()
    return _CACHE["nc"]


def kernel(features, q_w, q_b, k_w, k_b, v_w, v_b):
    nc = _get_nc()
    inv = 1.0 / (S * S)
    scale = float(KCH) ** -0.5
    qw_eff = np.ascontiguousarray(
        (q_w.T * (scale * inv)).astype(np.float32).reshape(CCH, 128, KCH))
    qb_eff = np.ascontiguousarray((q_b * scale).astype(np.float32))
    kw_eff = np.ascontiguousarray(
        (k_w.T * inv).astype(np.float32).reshape(CCH, 128, KCH))
    kb_eff = np.ascontiguousarray(k_b.astype(np.float32))
    vw_eff = np.ascontiguousarray(
        (v_w.T * inv).astype(np.float32).reshape(CCH, 128, C))
    vb_eff = np.ascontiguousarray(v_b.astype(np.float32))

    features = np.asarray(features, dtype=np.float32)
    in_maps = []
    for i in range(N_CORES):
        b, half = i // 2, i % 2
        in_maps.append({
            "xh": np.ascontiguousarray(
                features[b, :, half * HH:(half + 1) * HH, :]),
            "qw": qw_eff, "kw": kw_eff, "vw": vw_eff,
            "qb": qb_eff, "kb": kb_eff, "vb": vb_eff,
        })

    res = run_bass_kernel_spmd(nc, in_maps, list(range(N_CORES)))
    out = np.empty((B, C, H, W), dtype=np.float32)
    for i in range(N_CORES):
        b, half = i // 2, i % 2
        out[b, :, half * HH:(half + 1) * HH, :] = res.results[i]["out"]
    return out


# revision 9
# speedup vs baseline: 1.2205x; 1.2205x over previous
"""Trainium2 Bass kernel for BasicAttentionModule (pooled attention + residual).

Computation (per sample): 8x8 avg-pool -> 1x1-conv q/k/v over 1024 tokens ->
softmax attention -> nearest 8x upsample -> residual add.

Sharding: 2 cores per sample (batch 4 x 8 cores); each core owns 128 of the
256 rows.  Per core:
  phase 1  pool own half: H-direction sum rides on SWDGE accumulate-DMAs
           (1 MiB each, 2048 elem/partition CCE cap), W-direction on a DVE
           pairwise tree.
  gather   pooled tokens exchanged with the pair core via AllGather.
  attn     fp32 on the PE, tokens-on-partitions layout (no transposes):
           E[m,n] = exp(energy^T), denominator via all-ones matmul,
           out_small = vT.T @ E normalized on evacuation.
  phase 3  features stream in again on HWDGE (prefetched during attn),
           DVE adds the upsampled attention via a stride-0 broadcast
           operand, HWDGE stores.
"""

import numpy as np

import concourse.bass as bass
import concourse.mybir as mybir
import concourse.tile as tile
from concourse.bass_utils import run_bass_kernel_spmd

F32 = mybir.dt.float32

B, C, H, W = 4, 256, 256, 256
S = 8                      # pool stride
KCH = 32                   # key channels
N_CORES = 8
HH = H // 2                # 128 rows per core
HP, WP = HH // S, W // S   # 16 x 32 pooled grid per core half
NT = HP * WP               # 512 tokens owned per core
NTOK = 2 * NT              # 1024 tokens per sample
CCH = C // 128             # 2 channel chunks
NJ = NTOK // 128           # 8 token chunks (m on partitions)

_CACHE: dict = {}


def _split_multi_waits(nc):
    """walrus in this container accepts at most ONE sync-wait per
    instruction; hoist extra waits onto inserted NoOps (same engine,
    right before the instruction -> identical semantics)."""
    import json

    d = json.loads(mybir.module_to_json_string(nc.m))
    n = 0
    for fn in d["functions"]:
        for bb in fn["blocks"]:
            out = []
            for inst in bb.get("instructions", []):
                si = inst.get("sync_info")
                waits = (si or {}).get("on_wait") or []
                if len(waits) > 1:
                    for w in waits[:-1]:
                        n += 1
                        out.append({
                            "debug": inst.get("debug", 0),
                            "engine": inst["engine"],
                            "ins": [], "outs": [],
                            "name": f"I-wsplit-{n}",
                            "opcode": "NoOp",
                            "sync_info": {"on_update": [], "on_wait": [w]},
                        })
                    si["on_wait"] = [waits[-1]]
                out.append(inst)
            bb["instructions"] = out
    nc.m = mybir.module_from_json_string(json.dumps(d))
    return n


def _build(split_waits=True):
    nc = bass.Bass(num_devices=N_CORES)

    xh = nc.declare_dram_parameter("xh", [C, HH, W], F32, isOutput=False)
    qw = nc.declare_dram_parameter("qw", [CCH, 128, KCH], F32, isOutput=False)
    kw = nc.declare_dram_parameter("kw", [CCH, 128, KCH], F32, isOutput=False)
    vw = nc.declare_dram_parameter("vw", [CCH, 128, C], F32, isOutput=False)
    qb = nc.declare_dram_parameter("qb", [KCH], F32, isOutput=False)
    kb = nc.declare_dram_parameter("kb", [KCH], F32, isOutput=False)
    vb = nc.declare_dram_parameter("vb", [C], F32, isOutput=False)
    out = nc.declare_dram_parameter("out", [C, HH, W], F32, isOutput=True)

    with tile.TileContext(nc) as tc:
        with (
            tc.tile_pool(name="const", bufs=1) as constp,
            tc.tile_pool(name="wtree", bufs=2) as wtp,
            tc.tile_pool(name="attn", bufs=1) as attnp,
            tc.tile_pool(name="feat", bufs=5) as featp,
            tc.tile_pool(name="pqk", bufs=1, space="PSUM") as pqk,
            tc.tile_pool(name="pe", bufs=1, space="PSUM") as pep,
            tc.tile_pool(name="pacc", bufs=1, space="PSUM") as pacc,
            tc.tile_pool(name="dram", bufs=1, space="DRAM") as dram,
        ):
            # ---- constants / weights ----
            qw_sb = [constp.tile([128, KCH], F32, name=f"qw{k}") for k in range(CCH)]
            kw_sb = [constp.tile([128, KCH], F32, name=f"kw{k}") for k in range(CCH)]
            vw_sb = [constp.tile([128, C], F32, name=f"vw{k}") for k in range(CCH)]
            qb_sb = constp.tile([KCH, 1], F32, name="qb")
            kb_sb = constp.tile([KCH, 1], F32, name="kb")
            vb_row = constp.tile([1, C], F32, name="vb")
            for k in range(CCH):
                nc.scalar.dma_start(out=qw_sb[k][:], in_=qw[k])
                nc.scalar.dma_start(out=kw_sb[k][:], in_=kw[k])
                nc.scalar.dma_start(out=vw_sb[k][:], in_=vw[k])
            nc.scalar.dma_start(out=qb_sb[:], in_=qb[:])
            nc.scalar.dma_start(out=kb_sb[:], in_=kb[:])
            nc.scalar.dma_start(out=vb_row[:], in_=vb[:])
            ones_col = constp.tile([1, 128], F32, name="ones_col")
            ones128 = constp.tile([128, 128], F32, name="ones128")
            nc.vector.memset(ones_col[:], 1.0)
            nc.vector.memset(ones128[:], 1.0)

            xf_own = [constp.tile([128, NT], F32, name=f"xfo{k}") for k in range(CCH)]
            xf_full = [constp.tile([128, NTOK], F32, name=f"xff{k}") for k in range(CCH)]

            # ---- phase 1: stream 2 MiB slabs, pool on DVE pairwise trees ----
            for k in range(CCH):
                cs = slice(k * 128, (k + 1) * 128)
                for t in range(HP // 2):
                    rs = slice(t * 16, (t + 1) * 16)
                    feat = featp.tile([128, 16, W], F32, tag="feat", name="feat")
                    nc.sync.dma_start(out=feat[:], in_=xh[cs, rs, :])
                    l1 = wtp.tile([128, 8, W], F32, tag="l1", name="l1")
                    nc.vector.tensor_add(l1[:], feat[:, 0::2, :], feat[:, 1::2, :])
                    l2 = wtp.tile([128, 4, W], F32, tag="l2", name="l2")
                    nc.vector.tensor_add(l2[:], l1[:, 0::2, :], l1[:, 1::2, :])
                    l3 = wtp.tile([128, 2, W], F32, tag="l3", name="l3")
                    nc.vector.tensor_add(l3[:], l2[:, 0::2, :], l2[:, 1::2, :])
                    w1 = wtp.tile([128, 2, W // 2], F32, tag="w1", name="w1")
                    nc.vector.tensor_add(w1[:], l3[:, :, 0::2], l3[:, :, 1::2])
                    w2 = wtp.tile([128, 2, W // 4], F32, tag="w2", name="w2")
                    nc.vector.tensor_add(w2[:], w1[:, :, 0::2], w1[:, :, 1::2])
                    dst = xf_own[k][:, t * 2 * WP : (t + 1) * 2 * WP]
                    dst = dst.rearrange("c (i wp) -> c i wp", i=2)
                    nc.vector.tensor_add(dst, w2[:, :, 0::2], w2[:, :, 1::2])

            # ---- pair AllGather of pooled tokens ----
            cc_in = dram.tile([C, NT], F32, name="cc_in")
            cc_out = dram.tile([2, C, NT], F32, name="cc_out")
            for k in range(CCH):
                nc.gpsimd.dma_start(out=cc_in[k * 128:(k + 1) * 128, :],
                                    in_=xf_own[k][:])
            nc.gpsimd.collective_compute(
                "AllGather", mybir.AluOpType.bypass,
                replica_groups=[[0, 1], [2, 3], [4, 5], [6, 7]],
                ins=[cc_in.opt()], outs=[cc_out.opt()],
            )
            for k in range(CCH):
                src = cc_out[:, k * 128:(k + 1) * 128, :].rearrange("g c t -> c g t")
                dstf = xf_full[k].rearrange("c (g t) -> c g t", g=2)
                nc.scalar.dma_start(out=dstf, in_=src)

            # ---- q / k projections (q only needs own tokens) ----
            q_sb = attnp.tile([KCH, NT], F32, name="q_sb")
            psum_q = pqk.tile([KCH, NT], F32, tag="qk", name="psum_q")
            for k in range(CCH):
                nc.tensor.matmul(psum_q[:], qw_sb[k][:], xf_own[k][:],
                                 start=(k == 0), stop=(k == CCH - 1))
            nc.vector.tensor_scalar_add(q_sb[:], psum_q[:], qb_sb[:])

            k_sb = attnp.tile([KCH, NTOK], F32, name="k_sb")
            psum_k = pqk.tile([KCH, NTOK], F32, tag="qk", name="psum_k")
            for mh in range(2):
                ms = slice(mh * NT, (mh + 1) * NT)
                for k in range(CCH):
                    nc.tensor.matmul(psum_k[:, ms], kw_sb[k][:],
                                     xf_full[k][:, ms],
                                     start=(k == 0), stop=(k == CCH - 1))
            nc.vector.tensor_scalar_add(k_sb[:], psum_k[:], kb_sb[:])

            # ---- v^T (tokens on partitions), bias via K=1 ones matmul ----
            vt_sb = [attnp.tile([128, C], F32, name=f"vt{j}") for j in range(NJ)]
            for j in range(NJ):
                js = slice(j * 128, (j + 1) * 128)
                psum_vt = pep.tile([128, C], F32, tag="pvt", name="psum_vt")
                for k in range(CCH):
                    nc.tensor.matmul(psum_vt[:], xf_full[k][:, js], vw_sb[k][:],
                                     start=(k == 0), stop=False)
                nc.tensor.matmul(psum_vt[:], ones_col[:, :], vb_row[:],
                                 start=False, stop=True)
                nc.vector.tensor_copy(vt_sb[j][:], psum_vt[:])

            # ---- energies (E = exp(energy^T)) ----
            e_sb = [attnp.tile([128, NT], F32, name=f"e{j}") for j in range(NJ)]
            for j in range(NJ):
                js = slice(j * 128, (j + 1) * 128)
                psum_e = pep.tile([128, NT], F32, tag="pe", bufs=2, name="psum_e")
                nc.tensor.matmul(psum_e[:], k_sb[:, js], q_sb[:],
                                 start=True, stop=True)
                nc.scalar.activation(e_sb[j][:], psum_e[:],
                                     mybir.ActivationFunctionType.Exp)

            # ---- softmax denominator (all-ones matmul broadcasts it) ----
            recip = attnp.tile([128, NT], F32, name="recip")
            psum_den = pacc.tile([128, NT], F32, tag="den", name="psum_den")
            for j in range(NJ):
                nc.tensor.matmul(psum_den[:], ones128[:], e_sb[j][:],
                                 start=(j == 0), stop=(j == NJ - 1))
            nc.vector.reciprocal(recip[:], psum_den[:])

            # ---- attention output on the pooled grid, normalized ----
            os_sb = [attnp.tile([128, NT], F32, name=f"os{k}") for k in range(CCH)]
            for k in range(CCH):
                psum_os = pacc.tile([128, NT], F32, tag="pos", bufs=2,
                                    name="psum_os")
                for j in range(NJ):
                    nc.tensor.matmul(psum_os[:], vt_sb[j][:, k * 128:(k + 1) * 128],
                                     e_sb[j][:], start=(j == 0), stop=(j == NJ - 1))
                nc.vector.tensor_mul(os_sb[k][:], psum_os[:], recip[:])

            # ---- phase 3: second feature read + broadcast-add + store ----
            # feat loads queue on sync right after the phase-1 loads -> they
            # prefetch while the gather/attention runs; the DVE add runs
            # in-place on the slab, reading the upsampled attention values
            # through a stride-0 AP (no materialization), store from the
            # same slab on the scalar HWDGE ring.
            for k in range(CCH):
                cs = slice(k * 128, (k + 1) * 128)
                for t in range(HP // 2):
                    rs = slice(t * 16, (t + 1) * 16)
                    feat = featp.tile([128, 16, W], F32, tag="feat", name="feat")
                    nc.sync.dma_start(out=feat[:], in_=xh[cs, rs, :])
                    for hp in range(2):
                        src = bass.AP(os_sb[k].tensor,
                                      os_sb[k].offset + (t * 2 + hp) * WP,
                                      [list(os_sb[k].ap[0]),
                                       [0, S], [1, WP], [0, S]])
                        f4 = feat[:, hp * S:(hp + 1) * S, :].rearrange(
                            "c h (wp wr) -> c h wp wr", wr=S)
                        nc.vector.tensor_add(f4, f4, src)
                    nc.scalar.dma_start(out=out[cs, rs, :], in_=feat[:])

    if split_waits:
        _split_multi_waits(nc)
    return nc


def _get_nc():
    if "nc" not in _CACHE:
        _CACHE["nc"] = _build()
    return _CACHE["nc"]


def kernel(features, q_w, q_b, k_w, k_b, v_w, v_b):
    nc = _get_nc()
    inv = 1.0 / (S * S)
    scale = float(KCH) ** -0.5
    qw_eff = np.ascontiguousarray(
        (q_w.T * (scale * inv)).astype(np.float32).reshape(CCH, 128, KCH))
    qb_eff = np.ascontiguousarray((q_b * scale).astype(np.float32))
    kw_eff = np.ascontiguousarray(
        (k_w.T * inv).astype(np.float32).reshape(CCH, 128, KCH))
    kb_eff = np.ascontiguousarray(k_b.astype(np.float32))
    vw_eff = np.ascontiguousarray(
        (v_w.T * inv).astype(np.float32).reshape(CCH, 128, C))
    vb_eff = np.ascontiguousarray(v_b.astype(np.float32))

    features = np.asarray(features, dtype=np.float32)
    in_maps = []
    for i in range(N_CORES):
        b, half = i // 2, i % 2
        in_maps.append({
            "xh": np.ascontiguousarray(
                features[b, :, half * HH:(half + 1) * HH, :]),
            "qw": qw_eff, "kw": kw_eff, "vw": vw_eff,
            "qb": qb_eff, "kb": kb_eff, "vb": vb_eff,
        })

    res = run_bass_kernel_spmd(nc, in_maps, list(range(N_CORES)))
    out = np.empty((B, C, H, W), dtype=np.float32)
    for i in range(N_CORES):
        b, half = i // 2, i % 2
        out[b, :, half * HH:(half + 1) * HH, :] = res.results[i]["out"]
    return out
